# revision 4
# baseline (speedup 1.0000x reference)
"""APT encoder scatter kernel for TRN2 (8 NeuronCores, data-parallel over batch).

Problem: scatter patch tokens [B, P*BS, D] to a dense grid [B, H, W, T, BS, D]
per positions [B, P, 4] (rows y, x, size, t), broadcasting size-2 patches over
their 2x2 cell footprint.

Design (per core, one sample). Out row for cell (y, x, t) is 128*y + 4*x + t;
each row is BS*D = 2304 f32 = 9216 B. Every patch writes its base cell
(slot 0, always valid). The 2x2 broadcast of coarse (size==2) patches is done
with zero wasted DMA descriptors by compacting the coarse patches on device:

  1. base = 128*y + 4*x + t and is2 = (size >= 2), elementwise on positions.
  2. rank = exclusive prefix sum of is2 in patch order (free-dim scan +
     cross-partition exclusive prefix via a strictly-triangular matmul).
  3. Scatter (pid, base) records of coarse patches into a compact DRAM
     bounce at row=rank (fine patches land on a trash row).
  4. Load compact records back (4 tiles of 128), indirect-gather the coarse
     token rows, and issue 3 all-valid scatters per tile at base + {4, 128,
     132} (cells (y, x+1), (y+1, x), (y+1, x+1)).

Per-core HBM traffic: reads 23.6 (tokens) + 4.7 (coarse re-read) MB, writes
37.7 MB — no wasted bytes. The output param has pad rows (row >= 4096) that
absorb writes from unused compact slots if the input has fewer coarse
patches than compact capacity; the host slices them off.
"""

import numpy as np

import concourse.bass as bass
import concourse.bacc as bacc
import concourse.mybir as mybir
import concourse.tile as tile
from concourse.tile import add_dep_helper
from concourse.bass_utils import run_bass_kernel_spmd

B = 8
H, W, T, BS, D = 32, 32, 4, 3, 768
P = 2560             # patches per sample
ROW = BS * D         # 2304 f32 per patch/cell row
NCELL = H * W * T    # 4096 output rows per sample
PPART = 20           # patches per SBUF partition (2560 = 128 * 20)
NT = PPART           # scatter tiles per sample
NC2 = P // 5         # compact capacity for coarse patches (512 = exact count)
NG = NC2 // 128      # coarse gather tiles (4)
CMPROWS = 768        # compact bounce rows: 0..511 real, 600 trash, rest slop
TRASH = 600          # compact bounce row for fine-patch records
DUMMY = NCELL        # out row absorbing writes from unused compact slots
OUTROWS = 4352       # NCELL + pad >= DUMMY + 132 so slop writes stay inside

_CACHE = {}


def _build():
    nc = bacc.Bacc("TRN2", target_bir_lowering=False, debug=False, num_devices=B)
    tok = nc.declare_dram_parameter("tok", [P, ROW], mybir.dt.float32, isOutput=False)
    pos = nc.declare_dram_parameter("pos", [P, 4], mybir.dt.int32, isOutput=False)
    out = nc.declare_dram_parameter(
        "out", [OUTROWS, ROW], mybir.dt.float32, isOutput=True
    )

    i32 = mybir.dt.int32
    f32 = mybir.dt.float32
    Op = mybir.AluOpType

    with tile.TileContext(nc) as tc:
        with (
            tc.tile_pool(name="meta", bufs=1) as meta,
            tc.tile_pool(name="toks", bufs=4) as toks,
            tc.tile_pool(name="coarse", bufs=2) as cpool,
            tc.tile_pool(name="psum", bufs=1, space="PSUM") as psum,
            tc.tile_pool(name="dram", bufs=1, space="DRAM") as dpool,
        ):
            # ---- positions: partition p holds patches [20p, 20p+19] ----
            pos_sb = meta.tile([128, PPART * 4], i32)
            nc.sync.dma_start(
                out=pos_sb[:], in_=pos[:].rearrange("(p i) c -> p (i c)", p=128)
            )
            pos3 = pos_sb[:].rearrange("p (i c) -> p i c", c=4)
            y = pos3[:, :, 0]
            x = pos3[:, :, 1]
            s = pos3[:, :, 2]
            t = pos3[:, :, 3]

            # ---- base cell row and coarse flag ----
            base = meta.tile([128, PPART], i32)
            is2 = meta.tile([128, PPART], i32)
            nc.vector.tensor_scalar(
                out=base[:], in0=y, scalar1=128, scalar2=None, op0=Op.mult
            )
            nc.vector.scalar_tensor_tensor(
                out=base[:], in0=x, scalar=4, in1=base[:], op0=Op.mult, op1=Op.add
            )
            nc.vector.tensor_tensor(out=base[:], in0=base[:], in1=t, op=Op.add)
            nc.vector.tensor_scalar(
                out=is2[:], in0=s, scalar1=2, scalar2=None, op0=Op.is_ge
            )

            # ---- rank: exclusive prefix sum of is2 in patch order ----
            zeros = meta.tile([128, PPART], i32)
            nc.vector.memset(zeros[:], 0)
            scan = meta.tile([128, PPART], i32)
            nc.vector.tensor_tensor_scan(
                out=scan[:],
                data0=is2[:],
                data1=zeros[:],
                initial=0.0,
                op0=Op.add,
                op1=Op.add,
            )
            tot_f = meta.tile([128, 1], f32)
            nc.vector.tensor_copy(tot_f[:], scan[:, PPART - 1 : PPART])
            ones = meta.tile([128, 128], f32)
            nc.vector.memset(ones[:], 1.0)
            # M[k, p] = 1 iff p > k  ->  (lhsT.T @ tot)[p] = sum_{k<p} tot[k]
            trimat = meta.tile([128, 128], f32)
            nc.gpsimd.affine_select(
                out=trimat[:],
                in_=ones[:],
                pattern=[[1, 128]],
                compare_op=Op.is_gt,
                fill=0.0,
                base=0,
                channel_multiplier=-1,
            )
            part_off_ps = psum.tile([128, 1], f32)
            nc.tensor.matmul(
                out=part_off_ps[:], lhsT=trimat[:], rhs=tot_f[:], start=True, stop=True
            )
            part_off = meta.tile([128, 1], f32)
            nc.vector.tensor_copy(part_off[:], part_off_ps[:])

            rank = meta.tile([128, PPART], i32)
            nc.vector.tensor_tensor(out=rank[:], in0=scan[:], in1=is2[:], op=Op.subtract)
            nc.vector.tensor_scalar(
                out=rank[:], in0=rank[:], scalar1=part_off[:, 0:1], scalar2=NC2,
                op0=Op.add, op1=Op.min,
            )
            # record-scatter dest: coarse -> rank, fine -> TRASH
            dest = meta.tile([128, PPART], i32)
            nc.vector.tensor_scalar(
                out=dest[:], in0=rank[:], scalar1=-TRASH, scalar2=None, op0=Op.add
            )
            nc.vector.tensor_tensor(out=dest[:], in0=dest[:], in1=is2[:], op=Op.mult)
            nc.vector.tensor_scalar(
                out=dest[:], in0=dest[:], scalar1=TRASH, scalar2=None, op0=Op.add
            )

            # ---- records (pid, base) ----
            pid = meta.tile([128, PPART], i32)
            nc.gpsimd.iota(
                out=pid[:], pattern=[[1, PPART]], base=0, channel_multiplier=PPART
            )
            rec = meta.tile([128, PPART * 2], i32)
            rec3 = rec[:].rearrange("p (i c) -> p i c", c=2)
            nc.vector.tensor_copy(rec3[:, :, 0], pid[:])
            nc.vector.tensor_copy(rec3[:, :, 1], base[:])

            # ---- compact bounce: init to (pid=0, base=DUMMY) ----
            cmp = dpool.tile([CMPROWS, 2], i32)
            init_sb = meta.tile([128, CMPROWS * 2 // 128], i32)
            init3 = init_sb[:].rearrange("p (k c) -> p k c", c=2)
            nc.vector.memset(init_sb[:], 0)
            nc.vector.memset(init3[:, :, 1], DUMMY)
            init_dma = nc.sync.dma_start(
                out=cmp[:].rearrange("(p k) c -> p (k c)", p=128), in_=init_sb[:]
            )
            rec_insts = []
            for i in range(PPART):
                inst = nc.gpsimd.indirect_dma_start(
                    out=cmp[:],
                    out_offset=bass.IndirectOffsetOnAxis(ap=dest[:, i : i + 1], axis=0),
                    in_=rec3[:, i, :],
                    in_offset=None,
                )
                add_dep_helper(inst.ins, init_dma.ins, reason="cmp init before scatter")
                rec_insts.append(inst)

            # ---- slot-0: every patch writes its base cell ----
            tok_r = tok[:].rearrange("(p i) r -> p i r", i=PPART)
            for i in range(NT):
                tok_t = toks.tile([128, ROW], f32)
                nc.sync.dma_start(out=tok_t[:], in_=tok_r[:, i, :])
                nc.gpsimd.indirect_dma_start(
                    out=out[:],
                    out_offset=bass.IndirectOffsetOnAxis(ap=base[:, i : i + 1], axis=0),
                    in_=tok_t[:],
                    in_offset=None,
                )

            # ---- coarse patches: gather compacted rows, 3 scatters each ----
            for g in range(NG):
                cmp_sb = meta.tile([128, 2], i32, tag=f"cmp_sb{g}")
                ld = nc.sync.dma_start(
                    out=cmp_sb[:], in_=cmp[g * 128 : (g + 1) * 128, :]
                )
                for rinst in rec_insts:
                    add_dep_helper(ld.ins, rinst.ins, reason="records before load")
                coarse_t = cpool.tile([128, ROW], f32)
                nc.gpsimd.indirect_dma_start(
                    out=coarse_t[:],
                    out_offset=None,
                    in_=tok[:],
                    in_offset=bass.IndirectOffsetOnAxis(ap=cmp_sb[:, 0:1], axis=0),
                )
                offc = meta.tile([128, 3], i32, tag=f"offc{g}")
                for jj, cj in enumerate((4, 128, 132)):
                    nc.vector.tensor_scalar(
                        out=offc[:, jj : jj + 1],
                        in0=cmp_sb[:, 1:2],
                        scalar1=cj,
                        scalar2=None,
                        op0=Op.add,
                    )
                for jj in range(3):
                    nc.gpsimd.indirect_dma_start(
                        out=out[:],
                        out_offset=bass.IndirectOffsetOnAxis(
                            ap=offc[:, jj : jj + 1], axis=0
                        ),
                        in_=coarse_t[:],
                        in_offset=None,
                    )

    nc.compile()
    return nc


def _run(modality_tokens, positions, trace=False, tmpdir=None):
    nc = _CACHE.get("nc")
    if nc is None:
        nc = _CACHE["nc"] = _build()
    toks = np.ascontiguousarray(np.asarray(modality_tokens, dtype=np.float32)).reshape(
        B, P, ROW
    )
    poss = np.ascontiguousarray(np.asarray(positions, dtype=np.int32))
    in_maps = [{"tok": toks[b], "pos": poss[b]} for b in range(B)]
    res = run_bass_kernel_spmd(
        nc, in_maps, core_ids=list(range(B)), trace=trace, tmpdir=tmpdir
    )
    outf = np.stack([res.results[b]["out"][:NCELL] for b in range(B)])
    return outf.reshape(B, H, W, T, BS, D), res


def kernel(modality_tokens, positions):
    outf, _ = _run(modality_tokens, positions)
    return outf


# revision 8
# speedup vs baseline: 1.1150x; 1.1150x over previous
"""APT encoder scatter kernel for TRN2 (8 NeuronCores, data-parallel over batch).

Problem: scatter patch tokens [B, P*BS, D] to a dense grid [B, H, W, T, BS, D]
per positions [B, P, 4] (rows y, x, size, t), broadcasting size-2 patches over
their 2x2 cell footprint.

Design (per core, one sample). Out row for cell (y, x, t) is 128*y + 4*x + t;
each row is BS*D = 2304 f32 = 9216 B. Every patch writes its base cell
(slot 0, always valid). The 2x2 broadcast of coarse (size==2) patches is done
with zero wasted DMA descriptors by compacting the coarse patches on device:

  1. base = 128*y + 4*x + t and is2 = (size >= 2), elementwise on positions.
  2. rank = exclusive prefix sum of is2 in patch order (free-dim scan +
     cross-partition exclusive prefix via a strictly-triangular matmul).
  3. Scatter (pid, base) records of coarse patches into a compact DRAM
     bounce at row=rank (fine patches land on a trash row).
  4. Load compact records back (4 tiles of 128), indirect-gather the coarse
     token rows, and issue 3 all-valid scatters per tile at base + {4, 128,
     132} (cells (y, x+1), (y+1, x), (y+1, x+1)).

Per-core HBM traffic: reads 23.6 (tokens) + 4.7 (coarse re-read) MB, writes
37.7 MB — no wasted bytes. The output param has pad rows (row >= 4096) that
absorb writes from unused compact slots if the input has fewer coarse
patches than compact capacity; the host slices them off.
"""

import numpy as np

import concourse.bass as bass
import concourse.bacc as bacc
import concourse.mybir as mybir
import concourse.tile as tile
from concourse.tile import add_dep_helper
from concourse.bass_utils import run_bass_kernel_spmd

B = 8
H, W, T, BS, D = 32, 32, 4, 3, 768
P = 2560             # patches per sample
ROW = BS * D         # 2304 f32 per patch/cell row
NCELL = H * W * T    # 4096 output rows per sample
PPART = 20           # patches per SBUF partition (2560 = 128 * 20)
NT = PPART           # scatter tiles per sample
NC2 = P // 5         # compact capacity for coarse patches (512 = exact count)
NG = NC2 // 128      # coarse gather tiles (4)
CMPROWS = 768        # compact bounce rows: 0..511 real, 600 trash, rest slop
TRASH = 600          # compact bounce row for fine-patch records
DUMMY = NCELL        # out row absorbing writes from unused compact slots
OUTROWS = 4352       # NCELL + pad >= DUMMY + 132 so slop writes stay inside

_CACHE = {}


def _build():
    nc = bacc.Bacc("TRN2", target_bir_lowering=False, debug=False, num_devices=B)
    tok = nc.declare_dram_parameter("tok", [P, ROW], mybir.dt.float32, isOutput=False)
    pos = nc.declare_dram_parameter("pos", [P, 4], mybir.dt.int32, isOutput=False)
    out = nc.declare_dram_parameter(
        "out", [OUTROWS, ROW], mybir.dt.float32, isOutput=True
    )

    i32 = mybir.dt.int32
    f32 = mybir.dt.float32
    Op = mybir.AluOpType

    with tile.TileContext(nc) as tc:
        with (
            tc.tile_pool(name="meta", bufs=1) as meta,
            tc.tile_pool(name="toks", bufs=6) as toks,
            tc.tile_pool(name="coarse", bufs=2) as cpool,
            tc.tile_pool(name="psum", bufs=1, space="PSUM") as psum,
            tc.tile_pool(name="dram", bufs=1, space="DRAM") as dpool,
        ):
            # ---- positions: partition p holds patches [20p, 20p+19] ----
            pos_sb = meta.tile([128, PPART * 4], i32)
            nc.sync.dma_start(
                out=pos_sb[:], in_=pos[:].rearrange("(p i) c -> p (i c)", p=128)
            )
            pos3 = pos_sb[:].rearrange("p (i c) -> p i c", c=4)
            y = pos3[:, :, 0]
            x = pos3[:, :, 1]
            s = pos3[:, :, 2]
            t = pos3[:, :, 3]

            # ---- base cell row and coarse flag ----
            base = meta.tile([128, PPART], i32)
            is2 = meta.tile([128, PPART], i32)
            nc.vector.tensor_scalar(
                out=base[:], in0=y, scalar1=128, scalar2=None, op0=Op.mult
            )
            nc.vector.scalar_tensor_tensor(
                out=base[:], in0=x, scalar=4, in1=base[:], op0=Op.mult, op1=Op.add
            )
            nc.vector.tensor_tensor(out=base[:], in0=base[:], in1=t, op=Op.add)
            nc.vector.tensor_scalar(
                out=is2[:], in0=s, scalar1=2, scalar2=None, op0=Op.is_ge
            )

            # ---- rank: exclusive prefix sum of is2 in patch order ----
            zeros = meta.tile([128, PPART], i32)
            nc.vector.memset(zeros[:], 0)
            scan = meta.tile([128, PPART], i32)
            nc.vector.tensor_tensor_scan(
                out=scan[:],
                data0=is2[:],
                data1=zeros[:],
                initial=0.0,
                op0=Op.add,
                op1=Op.add,
            )
            tot_f = meta.tile([128, 1], f32)
            nc.vector.tensor_copy(tot_f[:], scan[:, PPART - 1 : PPART])
            ones = meta.tile([128, 128], f32)
            nc.vector.memset(ones[:], 1.0)
            # M[k, p] = 1 iff p > k  ->  (lhsT.T @ tot)[p] = sum_{k<p} tot[k]
            trimat = meta.tile([128, 128], f32)
            nc.gpsimd.affine_select(
                out=trimat[:],
                in_=ones[:],
                pattern=[[1, 128]],
                compare_op=Op.is_gt,
                fill=0.0,
                base=0,
                channel_multiplier=-1,
            )
            part_off_ps = psum.tile([128, 1], f32)
            nc.tensor.matmul(
                out=part_off_ps[:], lhsT=trimat[:], rhs=tot_f[:], start=True, stop=True
            )
            part_off = meta.tile([128, 1], f32)
            nc.vector.tensor_copy(part_off[:], part_off_ps[:])

            rank = meta.tile([128, PPART], i32)
            nc.vector.tensor_tensor(out=rank[:], in0=scan[:], in1=is2[:], op=Op.subtract)
            nc.vector.tensor_scalar(
                out=rank[:], in0=rank[:], scalar1=part_off[:, 0:1], scalar2=NC2,
                op0=Op.add, op1=Op.min,
            )
            # record-scatter dest: coarse -> rank, fine -> TRASH
            dest = meta.tile([128, PPART], i32)
            nc.vector.tensor_scalar(
                out=dest[:], in0=rank[:], scalar1=-TRASH, scalar2=None, op0=Op.add
            )
            nc.vector.tensor_tensor(out=dest[:], in0=dest[:], in1=is2[:], op=Op.mult)
            nc.vector.tensor_scalar(
                out=dest[:], in0=dest[:], scalar1=TRASH, scalar2=None, op0=Op.add
            )

            # ---- records (pid, base) ----
            pid = meta.tile([128, PPART], i32)
            nc.gpsimd.iota(
                out=pid[:], pattern=[[1, PPART]], base=0, channel_multiplier=PPART
            )
            rec = meta.tile([128, PPART * 2], i32)
            rec3 = rec[:].rearrange("p (i c) -> p i c", c=2)
            nc.vector.tensor_copy(rec3[:, :, 0], pid[:])
            nc.vector.tensor_copy(rec3[:, :, 1], base[:])

            # ---- compact bounce: init to (pid=0, base=DUMMY) ----
            cmp = dpool.tile([CMPROWS, 2], i32)
            init_sb = meta.tile([128, CMPROWS * 2 // 128], i32)
            init3 = init_sb[:].rearrange("p (k c) -> p k c", c=2)
            nc.vector.memset(init_sb[:], 0)
            nc.vector.memset(init3[:, :, 1], DUMMY)
            init_dma = nc.sync.dma_start(
                out=cmp[:].rearrange("(p k) c -> p (k c)", p=128), in_=init_sb[:]
            )
            # all indirect writes below target disjoint rows by construction;
            # collect them and downgrade their mutual WAW edges afterwards
            scatter_insts = []
            rec_insts = []

            # ---- slot-0 scatters interleaved with compaction record writes ----
            tok_r = tok[:].rearrange("(p i) r -> p i r", i=PPART)
            for i in range(NT):
                rinst = nc.gpsimd.indirect_dma_start(
                    out=cmp[:],
                    out_offset=bass.IndirectOffsetOnAxis(ap=dest[:, i : i + 1], axis=0),
                    in_=rec3[:, i, :],
                    in_offset=None,
                )
                add_dep_helper(rinst.ins, init_dma.ins, reason="cmp init before scatter")
                rec_insts.append(rinst)
                scatter_insts.append(rinst)

                tok_t = toks.tile([128, ROW], f32)
                nc.sync.dma_start(out=tok_t[:], in_=tok_r[:, i, :])
                sinst = nc.gpsimd.indirect_dma_start(
                    out=out[:],
                    out_offset=bass.IndirectOffsetOnAxis(ap=base[:, i : i + 1], axis=0),
                    in_=tok_t[:],
                    in_offset=None,
                )
                scatter_insts.append(sinst)

            # ---- coarse patches: gather compacted rows, 3 scatters each ----
            for g in range(NG):
                cmp_sb = meta.tile([128, 2], i32, tag=f"cmp_sb{g}")
                ld = nc.sync.dma_start(
                    out=cmp_sb[:], in_=cmp[g * 128 : (g + 1) * 128, :]
                )
                for rinst in rec_insts:
                    add_dep_helper(ld.ins, rinst.ins, reason="records before load")
                coarse_t = cpool.tile([128, ROW], f32)
                nc.gpsimd.indirect_dma_start(
                    out=coarse_t[:],
                    out_offset=None,
                    in_=tok[:],
                    in_offset=bass.IndirectOffsetOnAxis(ap=cmp_sb[:, 0:1], axis=0),
                )
                offc = meta.tile([128, 3], i32, tag=f"offc{g}")
                for jj, cj in enumerate((4, 128, 132)):
                    nc.vector.tensor_scalar(
                        out=offc[:, jj : jj + 1],
                        in0=cmp_sb[:, 1:2],
                        scalar1=cj,
                        scalar2=None,
                        op0=Op.add,
                    )
                for jj in range(3):
                    cinst = nc.gpsimd.indirect_dma_start(
                        out=out[:],
                        out_offset=bass.IndirectOffsetOnAxis(
                            ap=offc[:, jj : jj + 1], axis=0
                        ),
                        in_=coarse_t[:],
                        in_offset=None,
                    )
                    scatter_insts.append(cinst)

            # ---- downgrade spurious WAW edges between the indirect writes ----
            # Every scatter above hits rows no other scatter hits (the grid is
            # a partition; trash/dummy rows are write-only), so completion
            # ordering between them is unnecessary. Keep issue order (nosync)
            # so Tile's scheduler still has a total order on the engine.
            from concourse.instruction_name_ordered_set import (
                InstructionNameOrderedSet,
            )

            names = {s.ins.name for s in scatter_insts}
            for s in scatter_insts:
                ins = s.ins
                sync_deps = list(ins.sync_dependency_names())
                demote = [n for n in sync_deps if n in names]
                if demote:
                    ins.set_sync_dependencies(
                        InstructionNameOrderedSet(
                            [n for n in sync_deps if n not in names]
                        )
                    )
                    keep = list(ins.nosync_dependency_names())
                    ins.set_nosync_dependencies(
                        InstructionNameOrderedSet(keep + demote)
                    )

    nc.compile()
    return nc


def _run(modality_tokens, positions, trace=False, tmpdir=None):
    nc = _CACHE.get("nc")
    if nc is None:
        nc = _CACHE["nc"] = _build()
    toks = np.ascontiguousarray(np.asarray(modality_tokens, dtype=np.float32)).reshape(
        B, P, ROW
    )
    poss = np.ascontiguousarray(np.asarray(positions, dtype=np.int32))
    in_maps = [{"tok": toks[b], "pos": poss[b]} for b in range(B)]
    res = run_bass_kernel_spmd(
        nc, in_maps, core_ids=list(range(B)), trace=trace, tmpdir=tmpdir
    )
    outf = np.stack([res.results[b]["out"][:NCELL] for b in range(B)])
    return outf.reshape(B, H, W, T, BS, D), res


def kernel(modality_tokens, positions):
    outf, _ = _run(modality_tokens, positions)
    return outf
